# revision 2
# baseline (speedup 1.0000x reference)
"""Causal self-attention (T=2048, D=2048, H=16) on 8 Trainium2 NeuronCores.

Head-sharded tensor parallel, collective-free: each core computes its 2
heads' q/k/v projections and causal attention, then multiplies its
256-feature attention-output slice into the matching 256 rows of W_proj,
producing a FULL [D, T] partial of the output projection. The host sums
the 8 partials (the tensor-parallel all-reduce done host-side, where it
is free) — no on-device collective, no DRAM round-trip for the gathered
activations: the projection streams straight out of SBUF.

Layouts (all feature/d-major so the PE contracts along partitions):
  - xT      [D, T]  bf16 : x transposed (host-side; bf16 halves HBM traffic)
  - wqkvT   [D, 768] bf16: this core's W_attn rows (q0 q1 k0 k1 v0 v1), transposed
  - wpL     [256, D] f32r: W_proj columns for this core's features ( = lhsT )
  - qT/kT   [128, T] f32r per head (feature on partition)
  - v       [tok, 256] f32r (token on partition) so P.T@V needs no transpose
  - S_T     [tk, tq] so softmax sums run via an all-ones matmul on the PE
  - yT      [D, T]  bf16 partial output (host sums in fp32)

Attention is a single flat software pipeline across all (j, tk) tiles:
S/exp emission runs LOOKAHEAD tiles ahead of the ones/PV consumption —
across block boundaries — so the ACT round-trip never stalls the PE.
Projection of block j is emitted right after block j's normalize and
executes while block j+1's exps land. Diagonal tiles only compute
columns >= their causal offset (column-skip).
"""

import numpy as np
import ml_dtypes

import concourse.bacc as bacc
import concourse.bass_utils as bass_utils
import concourse.mybir as mybir
import concourse.tile as tile

T = 2048
D = 2048
H = 16
C = 128
N_CORES = 8
HPC = H // N_CORES          # heads per core = 2
FPC = HPC * C               # features per core = 256
TQB = 512                   # tq block (PSUM free-dim limit for fp32)
NTQ = T // TQB              # 4
NKT = T // 128              # 16 tk tiles
ND = D // 128               # 16 contraction tiles
SCALE = 1.0 / np.sqrt(np.float32(C))
LOOKAHEAD = 2

FR = mybir.dt.float32r
F32 = mybir.dt.float32
BF16 = mybir.dt.bfloat16

_NC_CACHE = {}


def build_nc(sim_single_core=False, reps=1, phases=3):
    key = ("sim" if sim_single_core else "nc") + f"_{reps}_{phases}"
    if key in _NC_CACHE:
        return _NC_CACHE[key]
    ndev = 1 if sim_single_core else N_CORES
    nc = bacc.Bacc("TRN2", target_bir_lowering=False, debug=False, num_devices=ndev)

    xT = nc.dram_tensor("xT", [D, T], BF16, kind="ExternalInput").ap()
    wqkvT = nc.dram_tensor("wqkvT", [D, 3 * FPC], BF16, kind="ExternalInput").ap()
    wpL = nc.dram_tensor("wpL", [FPC, D], FR, kind="ExternalInput").ap()
    # mask band: maskB[p, j] = 1.0 if p <= j - 384 else 0.0  (j in [0, 896))
    maskB = nc.dram_tensor("maskB", [128, 896], F32, kind="ExternalInput").ap()
    yT = nc.dram_tensor("yT", [D, T], BF16, kind="ExternalOutput").ap()

    with tile.TileContext(nc) as tc:
        with tc.tile_pool(name="persist", bufs=1) as pp, \
             tc.tile_pool(name="ptiles", bufs=12) as ppt, \
             tc.tile_pool(name="small", bufs=2) as smp, \
             tc.tile_pool(name="osb", bufs=4) as obp, \
             tc.tile_pool(name="ysb", bufs=4) as ybp, \
             tc.tile_pool(name="psA", bufs=4, space="PSUM") as psA, \
             tc.tile_pool(name="psB", bufs=2, space="PSUM") as psB, \
             tc.tile_pool(name="psC", bufs=2, space="PSUM") as psC:

            mask_sb = pp.tile([128, 896], F32, tag="mask")
            ones_f = pp.tile([128, 128], F32, tag="onesf")
            ones_r = pp.tile([128, 128], FR, tag="onesr")
            nc.vector.memset(ones_f[:], 1.0)
            nc.vector.tensor_copy(ones_r[:], ones_f[:])
            nc.sync.dma_start(mask_sb[:], maskB[:])

            for _rep in range(reps):
                emit_body(nc, tc, pp, ppt, smp, obp, ybp, psA, psB, psC,
                          xT, wqkvT, wpL, yT, mask_sb, ones_r, phases)

    nc.compile()
    _NC_CACHE[key] = nc
    return nc


def emit_body(nc, tc, pp, ppt, smp, obp, ybp, psA, psB, psC,
              xT, wqkvT, wpL, yT, mask_sb, ones_r, phases=3):
    # ---- phase 1: QKV projections ----
    # qkT layout: feature-block fb in {q_h0, q_h1, k_h0, k_h1} at cols
    # [fb*T, (fb+1)*T); v_sb: tok-tile tt at cols [tt*FPC, ...).
    qkT = pp.tile([128, 4 * T], FR, tag="qkT")               # 32KB/part
    v_sb = pp.tile([128, NKT * FPC], FR, tag="v")            # 16KB/part
    ph1_cm = tc.tile_pool(name="ph1", bufs=1)
    sp_cm = tc.tile_pool(name="stream", bufs=2)
    ph1 = ph1_cm.__enter__()
    sp = sp_cm.__enter__()
    w_sb = ph1.tile([128, ND * 3 * FPC], BF16, tag="wbig")   # 24KB/part
    xcols = []
    for tb in range(NTQ):
        xcols.append(sp.tile([128, ND * TQB], BF16, tag="xcol", name=f"xcol{tb}"))  # 16KB/part
    # interleave DMA emission so the first-needed tiles land first
    for t in range(ND):
        nc.sync.dma_start(
            w_sb[:, t * 3 * FPC:(t + 1) * 3 * FPC],
            wqkvT[t * 128:(t + 1) * 128, :])
        nc.sync.dma_start(
            xcols[0][:, t * TQB:(t + 1) * TQB],
            xT[t * 128:(t + 1) * 128, 0:TQB])
    for tb in range(NTQ):
        xcol = xcols[tb]
        if tb > 0:
            for t in range(ND):
                nc.sync.dma_start(
                    xcol[:, t * TQB:(t + 1) * TQB],
                    xT[t * 128:(t + 1) * 128, tb * TQB:(tb + 1) * TQB])
        for fb in range(4):
            ps = psA.tile([128, TQB], F32, tag="a")
            for t in range(ND):
                nc.tensor.matmul(
                    ps[:],
                    w_sb[:, t * 3 * FPC + fb * 128: t * 3 * FPC + fb * 128 + 128],
                    xcol[:, t * TQB:(t + 1) * TQB],
                    start=(t == 0), stop=(t == ND - 1))
            nc.vector.tensor_copy(
                qkT[:, fb * T + tb * TQB: fb * T + (tb + 1) * TQB], ps[:])
        for tt in range(4):
            tok = tb * 4 + tt
            ps = psB.tile([128, FPC], F32, tag="b")
            for t in range(ND):
                nc.tensor.matmul(
                    ps[:],
                    xcol[:, t * TQB + tt * 128: t * TQB + (tt + 1) * 128],
                    w_sb[:, t * 3 * FPC + 2 * FPC:(t + 1) * 3 * FPC],
                    start=(t == 0), stop=(t == ND - 1))
            nc.vector.tensor_copy(v_sb[:, tok * FPC:(tok + 1) * FPC], ps[:])

    sp_cm.__exit__(None, None, None)
    ph1_cm.__exit__(None, None, None)

    if phases == 1:
        for c in range(4):
            nc.sync.dma_start(yT[c * 128:(c + 1) * 128, :], qkT[:, c * T:(c + 1) * T])
        for c in range(2):
            nc.sync.dma_start(
                yT[(4 + c) * 128:(5 + c) * 128, :], v_sb[:, c * T:(c + 1) * T])
        return

    # ---- phase 2+3: flat attention pipeline + interleaved projection ----
    wpp_cm = tc.tile_pool(name="wproj", bufs=1)
    wpp = wpp_cm.__enter__()
    wp_sb = wpp.tile([128, HPC * D], FR, tag="wp")           # 16KB/part
    for hh in range(HPC):
        nc.sync.dma_start(wp_sb[:, hh * D:(hh + 1) * D], wpL[hh * 128:(hh + 1) * 128, :])

    # global tile list in execution order
    G = [(j, tk) for j in range(NTQ) for tk in range(4 * (j + 1))]
    p_tiles = {}
    proj_pending = []            # (j, ib, o_pair) tasks, drained 2/iteration

    def emit_s_exp(idx):
        j, tk = G[idx]
        d = max(0, tk * 128 - j * TQB)      # diagonal column-skip offset
        for h in range(HPC):
            qh = qkT[:, h * T:(h + 1) * T]
            kh = qkT[:, (2 + h) * T:(3 + h) * T]
            s_ps = psA.tile([128, TQB], F32, tag="a", name=f"s{j}_{h}_{tk}")
            nc.tensor.matmul(
                s_ps[:, d:TQB],
                kh[:, tk * 128:(tk + 1) * 128],
                qh[:, j * TQB + d:(j + 1) * TQB],
                start=True, stop=True)
            p_sb = ppt.tile([128, TQB], FR, tag="p", name=f"p{j}_{h}_{tk}")
            nc.scalar.activation(
                p_sb[:, d:TQB], s_ps[:, d:TQB], mybir.ActivationFunctionType.Exp,
                scale=float(SCALE))
            if tk * 128 - j * TQB >= 0:     # diagonal tile: causal mask
                nc.vector.tensor_mul(
                    p_sb[:, d:TQB], p_sb[:, d:TQB], mask_sb[:, 384:896 - d])
            p_tiles[h, idx] = p_sb

    for idx in range(min(LOOKAHEAD, len(G))):
        emit_s_exp(idx)
    ahead = min(LOOKAHEAD, len(G))

    def emit_proj_task(task_i, j, ib, o_pair):
        # y[ib-block, j-block] = sum_h wpL[h-tile].T @ o_h (contract 256 feats)
        # proj psum tiles share psA with the short-lived S tiles; sums and o
        # accumulators keep psB/psC to themselves, so the next block's
        # accumulation never waits on proj copies.
        ps = psA.tile([128, TQB], F32, tag="a", name=f"pj{j}_{ib}")
        nc.tensor.matmul(
            ps[:], wp_sb[:, 0 * D + ib * 128: 0 * D + (ib + 1) * 128],
            o_pair[0][:], start=True, stop=False)
        nc.tensor.matmul(
            ps[:], wp_sb[:, 1 * D + ib * 128: 1 * D + (ib + 1) * 128],
            o_pair[1][:], start=False, stop=True)
        y_sb = ybp.tile([128, TQB], BF16, tag="ysb", name=f"y{j}_{ib}")
        # split PSUM->SBUF evacuation across DVE and ACT so neither lags PE
        if task_i % 2 == 0:
            nc.vector.tensor_copy(y_sb[:], ps[:])
        else:
            nc.scalar.copy(y_sb[:], ps[:])
        nc.sync.dma_start(
            yT[ib * 128:(ib + 1) * 128, j * TQB:(j + 1) * TQB], y_sb[:])

    sum_ps = o_ps = None
    for idx in range(len(G)):
        j, tk = G[idx]
        n_tk = 4 * (j + 1)
        if tk == 0:
            sum_ps = [psB.tile([128, TQB], F32, tag="b", name=f"sum{j}_{h}")
                      for h in range(HPC)]
            o_ps = [psC.tile([128, TQB], F32, tag="c", name=f"ops{j}_{h}")
                    for h in range(HPC)]
        while ahead < len(G) and ahead - idx <= LOOKAHEAD:
            emit_s_exp(ahead)
            ahead += 1
        d = max(0, tk * 128 - j * TQB)
        for h in range(HPC):
            p_sb = p_tiles.pop((h, idx))
            nc.tensor.matmul(
                sum_ps[h][:, d:TQB], ones_r[:], p_sb[:, d:TQB],
                start=(tk == 0), stop=(tk == n_tk - 1))
            nc.tensor.matmul(
                o_ps[h][:, d:TQB],
                v_sb[:, tk * FPC + h * 128: tk * FPC + (h + 1) * 128],
                p_sb[:, d:TQB],
                start=(tk == 0), stop=(tk == n_tk - 1))
        # drain up to 2 pending proj tasks per consume iteration
        for _ in range(2):
            if proj_pending:
                ti, pj, pib, po = proj_pending.pop(0)
                emit_proj_task(ti, pj, pib, po)
        if tk == n_tk - 1:
            o_pair = []
            for h in range(HPC):
                inv_sb = smp.tile([128, TQB], F32, tag="inv", name=f"inv{j}_{h}")
                nc.vector.reciprocal(inv_sb[:], sum_ps[h][:])
                o_sb = obp.tile([128, TQB], FR, tag="osb", name=f"osb{j}_{h}")
                nc.vector.tensor_mul(o_sb[:], o_ps[h][:], inv_sb[:])
                if phases == 2:
                    nc.sync.dma_start(
                        yT[h * 128:(h + 1) * 128, j * TQB:(j + 1) * TQB], o_sb[:])
                o_pair.append(o_sb)
            if phases == 3:
                for ib in range(ND):
                    proj_pending.append((j * ND + ib, j, ib, o_pair))

    # drain the tail (the last block's projection)
    for ti, pj, pib, po in proj_pending:
        emit_proj_task(ti, pj, pib, po)

    wpp_cm.__exit__(None, None, None)


def make_mask_band() -> np.ndarray:
    p = np.arange(128)[:, None]
    j = np.arange(896)[None, :]
    return (p <= j - 384).astype(np.float32)


def prepare_in_maps(x, W_attn, W_proj):
    x = np.ascontiguousarray(np.asarray(x, dtype=np.float32))
    W_attn = np.ascontiguousarray(np.asarray(W_attn, dtype=np.float32))
    W_proj = np.ascontiguousarray(np.asarray(W_proj, dtype=np.float32))
    xT = np.ascontiguousarray(x.T.astype(ml_dtypes.bfloat16))
    mask = make_mask_band()
    in_maps = []
    for r in range(N_CORES):
        rows = slice(r * FPC, (r + 1) * FPC)
        w_qkv = np.concatenate(
            [W_attn[0 * D:][rows], W_attn[1 * D:][rows], W_attn[2 * D:][rows]],
            axis=0)                                   # [768, D]
        in_maps.append({
            "xT": xT,
            "wqkvT": np.ascontiguousarray(w_qkv.T.astype(ml_dtypes.bfloat16)),
            "wpL": np.ascontiguousarray(W_proj[:, rows].T),  # [256, D]
            "maskB": mask,
        })
    return in_maps


def postprocess(results) -> np.ndarray:
    acc = results[0]["yT"].astype(np.float32)
    for r in results[1:]:
        acc = acc + r["yT"].astype(np.float32)
    return np.ascontiguousarray(acc.T)


def expected_partial(inputs, core=0):
    """Reference value of one core's yT partial (for sim-side checking)."""
    x = np.asarray(inputs["x"], np.float32)
    W_attn = np.asarray(inputs["W_attn"], np.float32)
    W_proj = np.asarray(inputs["W_proj"], np.float32)
    qkv = x @ W_attn.T
    q, k, v = np.split(qkv, 3, axis=-1)
    outs = []
    for h in range(core * HPC, (core + 1) * HPC):
        qh = q[:, h * C:(h + 1) * C]
        kh = k[:, h * C:(h + 1) * C]
        vh = v[:, h * C:(h + 1) * C]
        s = (qh @ kh.T) / np.sqrt(np.float32(C))
        mask = np.tril(np.ones((T, T))) == 0
        s = np.where(mask, -np.inf, s)
        p = np.exp(s - s.max(axis=-1, keepdims=True))
        p /= p.sum(axis=-1, keepdims=True)
        outs.append(p.astype(np.float32) @ vh)
    out_r = np.concatenate(outs, axis=1)              # [T, 256]
    rows = slice(core * FPC, (core + 1) * FPC)
    y = out_r @ W_proj[:, rows].T                     # [T, D]
    return np.ascontiguousarray(y.T)                  # [D, T]


def kernel(x, W_attn, W_proj) -> np.ndarray:
    nc = build_nc()
    in_maps = prepare_in_maps(x, W_attn, W_proj)
    res = bass_utils.run_bass_kernel_spmd(
        nc, in_maps, core_ids=list(range(N_CORES)), trace=False)
    return postprocess(res.results)
